# revision 33
# baseline (speedup 1.0000x reference)
"""
Trainium2 Bass kernel for nn_Attention_335007449901 (sparse window attention).

Model (per image, eval mode):
  q = BN(conv1x1(x, wq)); k = BN(conv1x1(x, wk)); v = BN(conv1x1(x, wv))
  7x7 windows over the 112x112 image -> T=256 window tokens, token
  features = (channel, within-window position p) pairs.
  dots[i,j] = <q_i, k_j> * 0.125 ; attn = softmax_j ; out = attn @ v
  y = gelu(out); z = BN(conv1x1(y, wo) + bo); out = gelu(z + x)

Sharding: pure data parallel over batch, 4 images per core on 8 cores.

Implementation notes:
  * The window permute ('b c (h1 ws1) (w1 ws2) -> b (c ws1 ws2) (h1 w1)')
    is done ON THE HOST for the input, and inverted on the host for the
    output: the device sees x and writes out in position-major window
    layout [c, p*T + j], everything contiguous. The HW kernel does zero
    data reshuffling; the only copies are PSUM->SBUF casts.
  * BatchNorms folded into conv weights on the host; SCALE folded into q's
    path; k's bias drops (softmax shift invariance); v's bias passes
    through the attention average into the first gelu's bias; the final
    conv bias + BN fold into the last gelu's bias.
  * q and k never materialize: dots_T[j,i] = sum_p x_pj^T M x_pi with
    M = wk_f^T wq_f precomputed on the host, computed as u_p = M^T x_p
    then dots_T += u_p^T x_p. q's bias contributes a per-row term
    c[j] = sum_p (wk_f^T Bq) . x_p[:,j]; its M=1 matmuls are packed 4-up
    into 32-column strips of the PE array (col tiling), reduced with a
    selector-vector matmul, and added to dots via two rank-1 matmuls.
  * All matmul operands are bf16 (fp32 PSUM accumulation); x_winb is a
    contiguous bf16 copy of the win-layout image made by the otherwise
    idle GPSIMD engine.
  * dots are computed transposed so softmax normalization is a ones-vector
    matmul reduce; no max subtraction needed (|dots| < ~30).
  * The residual add is an identity matmul from x_winb accumulated into
    the out-conv PSUM, so the final gelu reads PSUM directly and writes
    its result IN PLACE into the (dead) x image buffer, contiguous.
"""

import numpy as np

IN_C = 128
HIDE_C = 256
HC2 = 128
OUT_C = 128
WS = 7
SCALE = 0.125
EPS = 1e-5
B, H, W = 32, 112, 112
HW = H * W          # 12544
H1 = H // WS        # 16
W1 = W // WS        # 16
T = H1 * W1         # 256 windows
NP = WS * WS        # 49 positions
NCORES = 8
BPC = B // NCORES   # images per core

F32 = np.float32


def build_bass_kernel(bpc=BPC):
    import concourse.bass as bass
    import concourse.tile as tile
    import concourse.mybir as mybir
    from concourse import bacc

    f32 = mybir.dt.float32
    bf16 = mybir.dt.bfloat16
    fp8 = mybir.dt.float8e4
    DR = mybir.MatmulPerfMode.DoubleRow
    AF = mybir.ActivationFunctionType

    nc = bacc.Bacc("TRN2", target_bir_lowering=False)

    x_d = nc.dram_tensor("x", [bpc, IN_C, HW], f32, kind="ExternalInput")
    m_d = nc.dram_tensor("m", [IN_C, IN_C], bf16, kind="ExternalInput")
    h_d = nc.dram_tensor("hcol", [IN_C, 1], bf16, kind="ExternalInput")
    ident_d = nc.dram_tensor("ident", [128, 128], bf16, kind="ExternalInput")
    wvT_d = nc.dram_tensor("wvT", [IN_C, HIDE_C], bf16, kind="ExternalInput")
    woT_d = nc.dram_tensor("woT", [HIDE_C, OUT_C], bf16, kind="ExternalInput")
    # packed per-partition fp32 bias columns: [Bv_lo, Bv_hi, Bo]
    bias_d = nc.dram_tensor("biases", [128, 3], f32, kind="ExternalInput")
    out_d = nc.dram_tensor("out", [bpc, OUT_C, HW], f32, kind="ExternalOutput")

    # position chunks: (start position, count), raster order
    groups = [(p, 2) for p in range(0, NP - 1, 2)] + [(NP - 1, 1)]

    with tile.TileContext(nc) as tc:
        with (
            tc.tile_pool(name="singles", bufs=1) as singles,
            tc.tile_pool(name="xpool", bufs=2) as xpool,
            tc.tile_pool(name="xwin", bufs=2) as xwin_pool,
            tc.tile_pool(name="u_sb", bufs=4) as u_sb_pool,
            tc.tile_pool(name="v_sb", bufs=13) as v_sb_pool,
            tc.tile_pool(name="g_sb", bufs=3) as g_sb_pool,
            tc.tile_pool(name="attn_sb", bufs=2) as attn_pool,
            tc.tile_pool(name="small_sb", bufs=2) as small_pool,
            tc.tile_pool(name="ps_work", bufs=2, space="PSUM") as ps_work,
            tc.tile_pool(name="ps_dots", bufs=1, space="PSUM") as ps_dots,
            tc.tile_pool(name="ps_av", bufs=3, space="PSUM") as ps_av,
            tc.tile_pool(name="ps_o", bufs=2, space="PSUM") as ps_o_pool,
        ):
            # ---- weights / constants (resident) ----
            m_sb = singles.tile([128, IN_C], bf16)
            nc.sync.dma_start(out=m_sb, in_=m_d.ap())
            h_sb = singles.tile([128, 1], bf16)
            nc.sync.dma_start(out=h_sb, in_=h_d.ap())
            ident = singles.tile([128, 128], bf16)
            nc.sync.dma_start(out=ident, in_=ident_d.ap())
            wvT = singles.tile([128, HIDE_C], bf16)
            nc.sync.dma_start(out=wvT, in_=wvT_d.ap())
            woT = singles.tile([128, 2, OUT_C], bf16)
            nc.sync.dma_start(
                out=woT, in_=woT_d.ap().rearrange("(kc p) m -> p kc m", kc=2)
            )
            biases = singles.tile([128, 3], f32)
            nc.sync.dma_start(out=biases, in_=bias_d.ap())
            bv_ap = [biases[:, 0:1], biases[:, 1:2]]
            bo_ap = biases[:, 2:3]

            ones_mat = singles.tile([128, 128], bf16)
            nc.vector.memset(ones_mat, 1.0)
            ones_row = singles.tile([1, T], bf16)
            nc.vector.memset(ones_row, 1.0)
            sel4 = singles.tile([128, 1], bf16)
            nc.vector.memset(sel4, 0.0)
            for t4 in range(4):
                nc.vector.memset(sel4[32 * t4:32 * t4 + 1, :], 1.0)

            for img in range(bpc):
                # ---- load win-layout x; bf16 copy mostly on GPSIMD ----
                x_img = xpool.tile([128, HW], f32, tag="ximg")
                NLD = 8
                for dc in range(NLD):
                    nc.sync.dma_start(
                        out=x_img[:, dc * (HW // NLD):(dc + 1) * (HW // NLD)],
                        in_=x_d.ap()[img, :, dc * (HW // NLD):(dc + 1) * (HW // NLD)])
                # dummy exp so walrus places the exp ACT-table load here,
                # off the softmax critical chain
                scratch = small_pool.tile([128, 1], f32, tag="scratch")
                nc.scalar.activation(scratch, biases[:, 0:1], AF.Exp)

                x_winb = xwin_pool.tile([128, NP * T], bf16, tag="xwin")
                NCH = 16
                for ch in range(NCH):
                    lo = ch * (HW // NCH)
                    hi = (ch + 1) * (HW // NCH)
                    # GPSIMD casts ~3x slower per element than DVE/ACT, but
                    # is otherwise idle; DVE/ACT take the first chunks so
                    # the dots pipeline starts early
                    if ch == 0:
                        nc.vector.tensor_copy(x_winb[:, lo:hi], x_img[:, lo:hi])
                    elif ch in (1, 8):
                        nc.scalar.activation(x_winb[:, lo:hi], x_img[:, lo:hi],
                                             AF.Copy, scale=1.0)
                    elif ch == 9:
                        nc.vector.tensor_copy(x_winb[:, lo:hi], x_img[:, lo:hi])
                    else:
                        nc.gpsimd.tensor_copy(x_winb[:, lo:hi], x_img[:, lo:hi])

                # ---- phase 1: dots_T accumulation over positions ----
                dots_t = ps_dots.tile([128, 512], f32, tag="dots", name="dots")
                dots = [dots_t[:, 0:T], dots_t[:, T:2 * T]]
                chunk_starts = list(range(0, NP, 2))   # 2 positions per chunk
                nchunks = len(chunk_starts)

                def u_conv(ci, p0):
                    npos = min(2, NP - p0)
                    N = npos * T
                    base = p0 * T
                    u_ps = ps_work.tile([128, 512], f32, tag="pwork")
                    nc.tensor.matmul(u_ps[:, :N], lhsT=m_sb,
                                     rhs=x_winb[:, base:base + N],
                                     start=True, stop=True)
                    u_sbt = u_sb_pool.tile([128, 512], bf16, tag="u")
                    nc.vector.tensor_copy(u_sbt[:, :N], u_ps[:, :N])
                    return u_sbt

                def dots_mms(ci, p0, u_sbt):
                    npos = min(2, NP - p0)
                    base = p0 * T
                    first = ci == 0
                    for pi in range(npos):
                        for jh in (0, 1):
                            nc.tensor.matmul(
                                dots[jh],
                                lhsT=u_sbt[:, pi * T + jh * 128:
                                           pi * T + jh * 128 + 128],
                                rhs=x_winb[:, base + pi * T:
                                           base + (pi + 1) * T],
                                start=first and pi == 0 and jh == 0,
                                stop=False,
                                skip_group_check=True)

                pend = []
                for ci, p0 in enumerate(chunk_starts):
                    u_sbt = u_conv(ci, p0)
                    if len(pend) >= 2:
                        dots_mms(*pend.pop(0))
                    pend.append((ci, p0, u_sbt))
                for pe_ in pend:
                    dots_mms(*pe_)
                # c[j] = sum_p h . x_p[:, j]. The M=1 matmuls are packed 4-up
                # into 32-column strips (col tiling), strip t accumulating
                # positions p%4==t concurrently; then a selector-vector
                # matmul reduces the strip rows and two rank-1 matmuls add
                # c into dots. c borrows an out-conv PSUM slot.
                c_row_ps = ps_o_pool.tile([128, 512], f32, tag="ops", name="cps")
                nc.vector.memset(c_row_ps[:, 0:T], 0.0)
                nstrip = [13, 12, 12, 12]
                seen = [0, 0, 0, 0]
                for p in range(NP):
                    t4 = p % 4
                    seen[t4] += 1
                    nc.tensor.matmul(c_row_ps[32 * t4:32 * t4 + 1, 0:T],
                                     lhsT=h_sb,
                                     rhs=x_winb[:, p * T:(p + 1) * T],
                                     start=seen[t4] == 1,
                                     stop=seen[t4] == nstrip[t4],
                                     tile_position=(0, 32 * t4),
                                     skip_group_check=True)
                c_all = small_pool.tile([128, T], bf16, tag="c4sb")
                nc.vector.tensor_copy(c_all, c_row_ps[:, 0:T])
                c_row_ps2 = ps_work.tile([1, T], f32, tag="pwork", name="cps2")
                nc.tensor.matmul(c_row_ps2, lhsT=sel4, rhs=c_all,
                                 start=True, stop=True)
                c_row = small_pool.tile([1, T], bf16, tag="csb")
                nc.vector.tensor_copy(c_row, c_row_ps2)
                for jh in (0, 1):
                    nc.tensor.matmul(
                        dots[jh], lhsT=c_row[:, jh * 128:jh * 128 + 128],
                        rhs=ones_row, start=False, stop=jh == 1,
                        skip_group_check=True)

                # ---- early v-convs (overlap the softmax chain on PE) ----
                def v_conv(g):
                    p0, cnt = g
                    vsb = []
                    for pi in range(cnt):
                        v_ps = ps_work.tile([128, 512], f32, tag="pwork")
                        for jc in (0, 1):
                            nc.tensor.matmul(
                                v_ps[:, jc * HIDE_C:(jc + 1) * HIDE_C],
                                lhsT=x_winb[:, (p0 + pi) * T + jc * 128:
                                            (p0 + pi) * T + jc * 128 + 128],
                                rhs=wvT,
                                start=True, stop=True)
                        v_sbt = v_sb_pool.tile([128, 512], bf16, tag="v")
                        nc.vector.tensor_copy(v_sbt, v_ps)
                        vsb.append(v_sbt)
                    return vsb

                NEARLY = 5
                early_v = [v_conv(g) for g in groups[:NEARLY]]

                # ---- softmax over j (= partitions of dots_T) ----
                attn_e = attn_pool.tile([128, 512], bf16, tag="attne",
                                        name="attne")
                nc.scalar.activation(attn_e, dots_t, AF.Exp)
                s_ps = ps_dots.tile([128, T], f32, tag="dots", name="ssum")
                for jc in (0, 1):
                    nc.tensor.matmul(s_ps, lhsT=ones_mat,
                                     rhs=attn_e[:, jc * T:(jc + 1) * T],
                                     start=jc == 0, stop=jc == 1)
                r_sb = small_pool.tile([128, T], f32, tag="rsb")
                nc.vector.reciprocal_approx_fast(r_sb, s_ps)
                attn2 = attn_pool.tile([128, 512], bf16, tag="attn",
                                       name="attn")
                attn = [attn2[:, 0:T], attn2[:, T:2 * T]]
                for jc in (0, 1):
                    nc.vector.tensor_mul(attn[jc],
                                         attn_e[:, jc * T:(jc + 1) * T], r_sb)

                # ---- phase 2: attention-average, out-conv, residual ----
                next_store = 0
                ST_CH = HW // 8
                vcache = dict(enumerate(early_v))
                for gi, g in enumerate(groups):
                    p0, cnt = g
                    N = cnt * T
                    base = p0 * T
                    if gi + NEARLY < len(groups):
                        vcache[gi + NEARLY] = v_conv(groups[gi + NEARLY])
                    vsb = vcache.pop(gi)

                    g_tiles = []
                    for kc in (0, 1):
                        av = ps_av.tile([128, 512], f32, tag="av", name=f"av{kc}")
                        for pi in range(cnt):
                            for jc in (0, 1):
                                nc.tensor.matmul(
                                    av[:, pi * T:(pi + 1) * T],
                                    lhsT=vsb[pi][:, jc * HIDE_C + kc * 128:
                                                  jc * HIDE_C + kc * 128 + 128],
                                    rhs=attn[jc],
                                    start=jc == 0, stop=jc == 1)
                        g_t = g_sb_pool.tile([128, 512], bf16, tag=f"g{kc}")
                        nc.scalar.activation(g_t[:, :N], av[:, :N], AF.Gelu,
                                             bias=bv_ap[kc], scale=1.0)
                        g_tiles.append(g_t)

                    # out conv + residual identity matmul, accumulated in
                    # PSUM; final gelu reads PSUM, writes the (dead) x
                    # image buffer in place, contiguous win layout
                    o_ps = ps_o_pool.tile([128, 512], f32, tag="ops")
                    for pi in range(cnt):
                        for kc in (0, 1):
                            nc.tensor.matmul(
                                o_ps[:, pi * T:(pi + 1) * T],
                                lhsT=woT[:, kc, :],
                                rhs=g_tiles[kc][:, pi * T:(pi + 1) * T],
                                start=kc == 0, stop=False)
                        nc.tensor.matmul(
                            o_ps[:, pi * T:(pi + 1) * T],
                            lhsT=ident,
                            rhs=x_winb[:, base + pi * T:base + (pi + 1) * T],
                            start=False, stop=True)
                    nc.scalar.activation(x_img[:, base:base + N],
                                         o_ps[:, :N], AF.Gelu,
                                         bias=bo_ap, scale=1.0)

                    # stream the store out as columns finalize: groups write
                    # x_img in raster order, so after group gi the first
                    # base+N columns are final
                    done = base + N
                    while next_store + ST_CH <= done:
                        nc.sync.dma_start(
                            out=out_d.ap()[img, :,
                                           next_store:next_store + ST_CH],
                            in_=x_img[:, next_store:next_store + ST_CH])
                        next_store += ST_CH

    nc.compile()
    return nc


def fold_params(wq, gq, bq, mq, vq, wk, gk, bk, mk, vk,
                wv, gv, bv, mv, vv, wo, bo, go, bbo, mo, vo):
    """Host-side BN/bias folding. Returns (M, h, ident, wvT, woT, biases)."""
    import ml_dtypes
    bf16 = ml_dtypes.bfloat16

    aq = gq / np.sqrt(vq + EPS)
    wq_f = (SCALE * aq)[:, None] * wq
    Bq = SCALE * (bq - aq * mq)

    ak = gk / np.sqrt(vk + EPS)
    wk_f = ak[:, None] * wk          # k bias drops (softmax shift invariance)

    M = wk_f.T @ wq_f                # dots_T = sum_p (M^T x_p)^T x_p
    hv = wk_f.T @ Bq                 # c[j] = sum_p hv . x_p[:, j]

    av = gv / np.sqrt(vv + EPS)
    wv_f = av[:, None] * wv
    Bv = bv - av * mv                # applied inside the first gelu

    ao = go / np.sqrt(vo + EPS)
    wo_f = ao[:, None] * wo
    Bo = ao * (bo - mo) + bbo        # conv bias + BN fold, inside last gelu

    biases = np.stack([Bv[:128], Bv[128:], Bo], axis=1).astype(F32)
    return (np.ascontiguousarray(M).astype(bf16),
            np.ascontiguousarray(hv[:, None]).astype(bf16),
            np.eye(128, dtype=bf16),
            np.ascontiguousarray(wv_f.T).astype(bf16),
            np.ascontiguousarray(wo_f.T).astype(bf16),
            biases)


_CACHED = {}


def _get_nc(bpc=BPC):
    if bpc not in _CACHED:
        _CACHED[bpc] = build_bass_kernel(bpc)
    return _CACHED[bpc]


def _to_win(x):
    """[n, c, H, W] image layout -> [n, c, p*T + j] win layout (host)."""
    n, c = x.shape[:2]
    x = x.reshape(n, c, H1, WS, W1, WS).transpose(0, 1, 3, 5, 2, 4)
    return np.ascontiguousarray(x.reshape(n, c, HW))


def _from_win(y):
    """[n, c, p*T + j] win layout -> [n, c, H, W] image layout (host)."""
    n, c = y.shape[:2]
    y = y.reshape(n, c, WS, WS, H1, W1).transpose(0, 1, 4, 2, 5, 3)
    return y.reshape(n, c, H, W)


def make_in_maps(inputs):
    x = np.asarray(inputs["x"], F32)
    m, hv, ident, wvT, woT, biases = fold_params(
        *[np.asarray(inputs[k], F32) for k in
          ("wq", "gq", "bq", "mq", "vq", "wk", "gk", "bk", "mk", "vk",
           "wv", "gv", "bv", "mv", "vv", "wo", "bo", "go", "bbo", "mo", "vo")]
    )
    in_maps = []
    for c in range(NCORES):
        xs = _to_win(x[c * BPC:(c + 1) * BPC])
        in_maps.append({"x": xs, "m": m, "hcol": hv, "ident": ident,
                        "wvT": wvT, "woT": woT, "biases": biases})
    return in_maps


def kernel(**inputs):
    from concourse.bass_utils import run_bass_kernel_spmd

    in_maps = make_in_maps(inputs)
    nc = _get_nc(BPC)
    res = run_bass_kernel_spmd(nc, in_maps, list(range(NCORES)))
    outs = [_from_win(res.results[c]["out"].reshape(BPC, OUT_C, HW))
            for c in range(NCORES)]
    return np.concatenate(outs, axis=0)


# revision 34
# speedup vs baseline: 1.0330x; 1.0330x over previous
"""
Trainium2 Bass kernel for nn_Attention_335007449901 (sparse window attention).

Model (per image, eval mode):
  q = BN(conv1x1(x, wq)); k = BN(conv1x1(x, wk)); v = BN(conv1x1(x, wv))
  7x7 windows over the 112x112 image -> T=256 window tokens, token
  features = (channel, within-window position p) pairs.
  dots[i,j] = <q_i, k_j> * 0.125 ; attn = softmax_j ; out = attn @ v
  y = gelu(out); z = BN(conv1x1(y, wo) + bo); out = gelu(z + x)

Sharding: pure data parallel over batch, 4 images per core on 8 cores.

Implementation notes:
  * The window permute ('b c (h1 ws1) (w1 ws2) -> b (c ws1 ws2) (h1 w1)')
    is done ON THE HOST for the input, and inverted on the host for the
    output: the device sees x and writes out in position-major window
    layout [c, p*T + j], everything contiguous. The HW kernel does zero
    data reshuffling; the only copies are PSUM->SBUF casts.
  * BatchNorms folded into conv weights on the host; SCALE folded into q's
    path; k's bias drops (softmax shift invariance); v's bias passes
    through the attention average into the first gelu's bias; the final
    conv bias + BN fold into the last gelu's bias.
  * q and k never materialize: dots_T[j,i] = sum_p x_pj^T M x_pi with
    M = wk_f^T wq_f precomputed on the host, computed as u_p = M^T x_p
    then dots_T += u_p^T x_p. q's bias contributes a per-row term
    c[j] = sum_p (wk_f^T Bq) . x_p[:,j]; its M=1 matmuls are packed 4-up
    into 32-column strips of the PE array (col tiling), reduced with a
    selector-vector matmul, and added to dots via two rank-1 matmuls.
  * All matmul operands are bf16 (fp32 PSUM accumulation); x_winb is a
    contiguous bf16 copy of the win-layout image made by the otherwise
    idle GPSIMD engine.
  * dots are computed transposed so softmax normalization is a ones-vector
    matmul reduce; no max subtraction needed (|dots| < ~30).
  * The residual add is an identity matmul from x_winb accumulated into
    the out-conv PSUM, so the final gelu reads PSUM directly and writes
    its result IN PLACE into the (dead) x image buffer, contiguous.
"""

import numpy as np

IN_C = 128
HIDE_C = 256
HC2 = 128
OUT_C = 128
WS = 7
SCALE = 0.125
EPS = 1e-5
B, H, W = 32, 112, 112
HW = H * W          # 12544
H1 = H // WS        # 16
W1 = W // WS        # 16
T = H1 * W1         # 256 windows
NP = WS * WS        # 49 positions
NCORES = 8
BPC = B // NCORES   # images per core

F32 = np.float32


def build_bass_kernel(bpc=BPC):
    import concourse.bass as bass
    import concourse.tile as tile
    import concourse.mybir as mybir
    from concourse import bacc

    f32 = mybir.dt.float32
    bf16 = mybir.dt.bfloat16
    fp8 = mybir.dt.float8e4
    DR = mybir.MatmulPerfMode.DoubleRow
    AF = mybir.ActivationFunctionType

    nc = bacc.Bacc("TRN2", target_bir_lowering=False)

    x_d = nc.dram_tensor("x", [bpc, IN_C, HW], f32, kind="ExternalInput")
    m_d = nc.dram_tensor("m", [IN_C, IN_C], bf16, kind="ExternalInput")
    h_d = nc.dram_tensor("hcol", [IN_C, 1], bf16, kind="ExternalInput")
    ident_d = nc.dram_tensor("ident", [128, 128], bf16, kind="ExternalInput")
    wvT_d = nc.dram_tensor("wvT", [IN_C, HIDE_C], bf16, kind="ExternalInput")
    woT_d = nc.dram_tensor("woT", [HIDE_C, OUT_C], bf16, kind="ExternalInput")
    # packed per-partition fp32 bias columns: [Bv_lo, Bv_hi, Bo]
    bias_d = nc.dram_tensor("biases", [128, 3], f32, kind="ExternalInput")
    out_d = nc.dram_tensor("out", [bpc, OUT_C, HW], f32, kind="ExternalOutput")

    # position chunks: (start position, count), raster order
    groups = [(p, 2) for p in range(0, NP - 1, 2)] + [(NP - 1, 1)]

    with tile.TileContext(nc) as tc:
        with (
            tc.tile_pool(name="singles", bufs=1) as singles,
            tc.tile_pool(name="xpool", bufs=2) as xpool,
            tc.tile_pool(name="xwin", bufs=2) as xwin_pool,
            tc.tile_pool(name="u_sb", bufs=4) as u_sb_pool,
            tc.tile_pool(name="v_sb", bufs=10) as v_sb_pool,
            tc.tile_pool(name="g_sb", bufs=3) as g_sb_pool,
            tc.tile_pool(name="attn_sb", bufs=2) as attn_pool,
            tc.tile_pool(name="small_sb", bufs=2) as small_pool,
            tc.tile_pool(name="ps_work", bufs=2, space="PSUM") as ps_work,
            tc.tile_pool(name="ps_dots", bufs=1, space="PSUM") as ps_dots,
            tc.tile_pool(name="ps_av", bufs=3, space="PSUM") as ps_av,
            tc.tile_pool(name="ps_o", bufs=2, space="PSUM") as ps_o_pool,
        ):
            # ---- weights / constants (resident) ----
            m_sb = singles.tile([128, IN_C], bf16)
            nc.sync.dma_start(out=m_sb, in_=m_d.ap())
            h_sb = singles.tile([128, 1], bf16)
            nc.sync.dma_start(out=h_sb, in_=h_d.ap())
            ident = singles.tile([128, 128], bf16)
            nc.sync.dma_start(out=ident, in_=ident_d.ap())
            wvT = singles.tile([128, HIDE_C], bf16)
            nc.sync.dma_start(out=wvT, in_=wvT_d.ap())
            woT = singles.tile([128, 2, OUT_C], bf16)
            nc.sync.dma_start(
                out=woT, in_=woT_d.ap().rearrange("(kc p) m -> p kc m", kc=2)
            )
            biases = singles.tile([128, 3], f32)
            nc.sync.dma_start(out=biases, in_=bias_d.ap())
            bv_ap = [biases[:, 0:1], biases[:, 1:2]]
            bo_ap = biases[:, 2:3]

            ones_mat = singles.tile([128, 128], bf16)
            nc.vector.memset(ones_mat, 1.0)
            ones_row = singles.tile([1, T], bf16)
            nc.vector.memset(ones_row, 1.0)
            sel4 = singles.tile([128, 1], bf16)
            nc.vector.memset(sel4, 0.0)
            for t4 in range(4):
                nc.vector.memset(sel4[32 * t4:32 * t4 + 1, :], 1.0)

            for img in range(bpc):
                # ---- load win-layout x; bf16 copy mostly on GPSIMD ----
                x_img = xpool.tile([128, HW], f32, tag="ximg")
                NLD = 8
                for dc in range(NLD):
                    nc.sync.dma_start(
                        out=x_img[:, dc * (HW // NLD):(dc + 1) * (HW // NLD)],
                        in_=x_d.ap()[img, :, dc * (HW // NLD):(dc + 1) * (HW // NLD)])
                # dummy exp so walrus places the exp ACT-table load here,
                # off the softmax critical chain
                scratch = small_pool.tile([128, 1], f32, tag="scratch")
                nc.scalar.activation(scratch, biases[:, 0:1], AF.Exp)

                x_winb = xwin_pool.tile([128, NP * T], bf16, tag="xwin")
                NCH = 16
                for ch in range(NCH):
                    lo = ch * (HW // NCH)
                    hi = (ch + 1) * (HW // NCH)
                    # GPSIMD casts ~3x slower per element than DVE/ACT, but
                    # is otherwise idle; DVE/ACT take a few chunks to keep
                    # the dots pipeline fed early
                    if ch % 8 < 5:
                        nc.gpsimd.tensor_copy(x_winb[:, lo:hi], x_img[:, lo:hi])
                    elif ch % 8 < 7:
                        nc.scalar.activation(x_winb[:, lo:hi], x_img[:, lo:hi],
                                             AF.Copy, scale=1.0)
                    else:
                        nc.vector.tensor_copy(x_winb[:, lo:hi], x_img[:, lo:hi])

                # ---- phase 1: dots_T accumulation over positions ----
                dots_t = ps_dots.tile([128, 512], f32, tag="dots", name="dots")
                dots = [dots_t[:, 0:T], dots_t[:, T:2 * T]]
                chunk_starts = list(range(0, NP, 2))   # 2 positions per chunk
                nchunks = len(chunk_starts)

                def u_conv(ci, p0):
                    npos = min(2, NP - p0)
                    N = npos * T
                    base = p0 * T
                    u_ps = ps_work.tile([128, 512], f32, tag="pwork")
                    nc.tensor.matmul(u_ps[:, :N], lhsT=m_sb,
                                     rhs=x_winb[:, base:base + N],
                                     start=True, stop=True)
                    u_sbt = u_sb_pool.tile([128, 512], bf16, tag="u")
                    nc.vector.tensor_copy(u_sbt[:, :N], u_ps[:, :N])
                    return u_sbt

                def dots_mms(ci, p0, u_sbt):
                    npos = min(2, NP - p0)
                    base = p0 * T
                    first = ci == 0
                    for pi in range(npos):
                        for jh in (0, 1):
                            nc.tensor.matmul(
                                dots[jh],
                                lhsT=u_sbt[:, pi * T + jh * 128:
                                           pi * T + jh * 128 + 128],
                                rhs=x_winb[:, base + pi * T:
                                           base + (pi + 1) * T],
                                start=first and pi == 0 and jh == 0,
                                stop=False,
                                skip_group_check=True)

                pend = []
                for ci, p0 in enumerate(chunk_starts):
                    u_sbt = u_conv(ci, p0)
                    if len(pend) >= 2:
                        dots_mms(*pend.pop(0))
                    pend.append((ci, p0, u_sbt))
                for pe_ in pend:
                    dots_mms(*pe_)
                # c[j] = sum_p h . x_p[:, j]. The M=1 matmuls are packed 4-up
                # into 32-column strips (col tiling), strip t accumulating
                # positions p%4==t concurrently; then a selector-vector
                # matmul reduces the strip rows and two rank-1 matmuls add
                # c into dots. c borrows an out-conv PSUM slot.
                c_row_ps = ps_o_pool.tile([128, 512], f32, tag="ops", name="cps")
                nc.vector.memset(c_row_ps[:, 0:T], 0.0)
                nstrip = [13, 12, 12, 12]
                seen = [0, 0, 0, 0]
                for p in range(NP):
                    t4 = p % 4
                    seen[t4] += 1
                    nc.tensor.matmul(c_row_ps[32 * t4:32 * t4 + 1, 0:T],
                                     lhsT=h_sb,
                                     rhs=x_winb[:, p * T:(p + 1) * T],
                                     start=seen[t4] == 1,
                                     stop=seen[t4] == nstrip[t4],
                                     tile_position=(0, 32 * t4),
                                     skip_group_check=True)
                c_all = small_pool.tile([128, T], bf16, tag="c4sb")
                nc.vector.tensor_copy(c_all, c_row_ps[:, 0:T])
                c_row_ps2 = ps_work.tile([1, T], f32, tag="pwork", name="cps2")
                nc.tensor.matmul(c_row_ps2, lhsT=sel4, rhs=c_all,
                                 start=True, stop=True)
                c_row = small_pool.tile([1, T], bf16, tag="csb")
                nc.vector.tensor_copy(c_row, c_row_ps2)
                for jh in (0, 1):
                    nc.tensor.matmul(
                        dots[jh], lhsT=c_row[:, jh * 128:jh * 128 + 128],
                        rhs=ones_row, start=False, stop=jh == 1,
                        skip_group_check=True)

                # ---- early v-convs (overlap the softmax chain on PE) ----
                def v_conv(g):
                    p0, cnt = g
                    vsb = []
                    for pi in range(cnt):
                        v_ps = ps_work.tile([128, 512], f32, tag="pwork")
                        for jc in (0, 1):
                            nc.tensor.matmul(
                                v_ps[:, jc * HIDE_C:(jc + 1) * HIDE_C],
                                lhsT=x_winb[:, (p0 + pi) * T + jc * 128:
                                            (p0 + pi) * T + jc * 128 + 128],
                                rhs=wvT,
                                start=True, stop=True)
                        v_sbt = v_sb_pool.tile([128, 512], bf16, tag="v")
                        nc.vector.tensor_copy(v_sbt, v_ps)
                        vsb.append(v_sbt)
                    return vsb

                NEARLY = 3
                early_v = [v_conv(g) for g in groups[:NEARLY]]

                # ---- softmax over j (= partitions of dots_T) ----
                attn_e = attn_pool.tile([128, 512], bf16, tag="attne",
                                        name="attne")
                nc.scalar.activation(attn_e, dots_t, AF.Exp)
                s_ps = ps_dots.tile([128, T], f32, tag="dots", name="ssum")
                for jc in (0, 1):
                    nc.tensor.matmul(s_ps, lhsT=ones_mat,
                                     rhs=attn_e[:, jc * T:(jc + 1) * T],
                                     start=jc == 0, stop=jc == 1)
                r_sb = small_pool.tile([128, T], f32, tag="rsb")
                nc.vector.reciprocal_approx_fast(r_sb, s_ps)
                attn2 = attn_pool.tile([128, 512], bf16, tag="attn",
                                       name="attn")
                attn = [attn2[:, 0:T], attn2[:, T:2 * T]]
                for jc in (0, 1):
                    nc.vector.tensor_mul(attn[jc],
                                         attn_e[:, jc * T:(jc + 1) * T], r_sb)

                # ---- phase 2: attention-average, out-conv, residual ----
                next_store = 0
                ST_CH = HW // 8
                vcache = dict(enumerate(early_v))
                for gi, g in enumerate(groups):
                    p0, cnt = g
                    N = cnt * T
                    base = p0 * T
                    if gi + NEARLY < len(groups):
                        vcache[gi + NEARLY] = v_conv(groups[gi + NEARLY])
                    vsb = vcache.pop(gi)

                    g_tiles = []
                    for kc in (0, 1):
                        av = ps_av.tile([128, 512], f32, tag="av", name=f"av{kc}")
                        for pi in range(cnt):
                            for jc in (0, 1):
                                nc.tensor.matmul(
                                    av[:, pi * T:(pi + 1) * T],
                                    lhsT=vsb[pi][:, jc * HIDE_C + kc * 128:
                                                  jc * HIDE_C + kc * 128 + 128],
                                    rhs=attn[jc],
                                    start=jc == 0, stop=jc == 1)
                        g_t = g_sb_pool.tile([128, 512], bf16, tag=f"g{kc}")
                        nc.scalar.activation(g_t[:, :N], av[:, :N], AF.Gelu,
                                             bias=bv_ap[kc], scale=1.0)
                        g_tiles.append(g_t)

                    # out conv + residual identity matmul, accumulated in
                    # PSUM; final gelu reads PSUM, writes the (dead) x
                    # image buffer in place, contiguous win layout
                    o_ps = ps_o_pool.tile([128, 512], f32, tag="ops")
                    for pi in range(cnt):
                        for kc in (0, 1):
                            nc.tensor.matmul(
                                o_ps[:, pi * T:(pi + 1) * T],
                                lhsT=woT[:, kc, :],
                                rhs=g_tiles[kc][:, pi * T:(pi + 1) * T],
                                start=kc == 0, stop=False)
                        nc.tensor.matmul(
                            o_ps[:, pi * T:(pi + 1) * T],
                            lhsT=ident,
                            rhs=x_winb[:, base + pi * T:base + (pi + 1) * T],
                            start=False, stop=True)
                    nc.scalar.activation(x_img[:, base:base + N],
                                         o_ps[:, :N], AF.Gelu,
                                         bias=bo_ap, scale=1.0)

                    # stream the store out as columns finalize: groups write
                    # x_img in raster order, so after group gi the first
                    # base+N columns are final
                    done = base + N
                    while next_store + ST_CH <= done:
                        nc.sync.dma_start(
                            out=out_d.ap()[img, :,
                                           next_store:next_store + ST_CH],
                            in_=x_img[:, next_store:next_store + ST_CH])
                        next_store += ST_CH

    nc.compile()
    return nc


def fold_params(wq, gq, bq, mq, vq, wk, gk, bk, mk, vk,
                wv, gv, bv, mv, vv, wo, bo, go, bbo, mo, vo):
    """Host-side BN/bias folding. Returns (M, h, ident, wvT, woT, biases)."""
    import ml_dtypes
    bf16 = ml_dtypes.bfloat16

    aq = gq / np.sqrt(vq + EPS)
    wq_f = (SCALE * aq)[:, None] * wq
    Bq = SCALE * (bq - aq * mq)

    ak = gk / np.sqrt(vk + EPS)
    wk_f = ak[:, None] * wk          # k bias drops (softmax shift invariance)

    M = wk_f.T @ wq_f                # dots_T = sum_p (M^T x_p)^T x_p
    hv = wk_f.T @ Bq                 # c[j] = sum_p hv . x_p[:, j]

    av = gv / np.sqrt(vv + EPS)
    wv_f = av[:, None] * wv
    Bv = bv - av * mv                # applied inside the first gelu

    ao = go / np.sqrt(vo + EPS)
    wo_f = ao[:, None] * wo
    Bo = ao * (bo - mo) + bbo        # conv bias + BN fold, inside last gelu

    biases = np.stack([Bv[:128], Bv[128:], Bo], axis=1).astype(F32)
    return (np.ascontiguousarray(M).astype(bf16),
            np.ascontiguousarray(hv[:, None]).astype(bf16),
            np.eye(128, dtype=bf16),
            np.ascontiguousarray(wv_f.T).astype(bf16),
            np.ascontiguousarray(wo_f.T).astype(bf16),
            biases)


_CACHED = {}


def _get_nc(bpc=BPC):
    if bpc not in _CACHED:
        _CACHED[bpc] = build_bass_kernel(bpc)
    return _CACHED[bpc]


def _to_win(x):
    """[n, c, H, W] image layout -> [n, c, p*T + j] win layout (host)."""
    n, c = x.shape[:2]
    x = x.reshape(n, c, H1, WS, W1, WS).transpose(0, 1, 3, 5, 2, 4)
    return np.ascontiguousarray(x.reshape(n, c, HW))


def _from_win(y):
    """[n, c, p*T + j] win layout -> [n, c, H, W] image layout (host)."""
    n, c = y.shape[:2]
    y = y.reshape(n, c, WS, WS, H1, W1).transpose(0, 1, 4, 2, 5, 3)
    return y.reshape(n, c, H, W)


def make_in_maps(inputs):
    x = np.asarray(inputs["x"], F32)
    m, hv, ident, wvT, woT, biases = fold_params(
        *[np.asarray(inputs[k], F32) for k in
          ("wq", "gq", "bq", "mq", "vq", "wk", "gk", "bk", "mk", "vk",
           "wv", "gv", "bv", "mv", "vv", "wo", "bo", "go", "bbo", "mo", "vo")]
    )
    in_maps = []
    for c in range(NCORES):
        xs = _to_win(x[c * BPC:(c + 1) * BPC])
        in_maps.append({"x": xs, "m": m, "hcol": hv, "ident": ident,
                        "wvT": wvT, "woT": woT, "biases": biases})
    return in_maps


def kernel(**inputs):
    from concourse.bass_utils import run_bass_kernel_spmd

    in_maps = make_in_maps(inputs)
    nc = _get_nc(BPC)
    res = run_bass_kernel_spmd(nc, in_maps, list(range(NCORES)))
    outs = [_from_win(res.results[c]["out"].reshape(BPC, OUT_C, HW))
            for c in range(NCORES)]
    return np.concatenate(outs, axis=0)


# revision 35
# speedup vs baseline: 1.0418x; 1.0085x over previous
"""
Trainium2 Bass kernel for nn_Attention_335007449901 (sparse window attention).

Model (per image, eval mode):
  q = BN(conv1x1(x, wq)); k = BN(conv1x1(x, wk)); v = BN(conv1x1(x, wv))
  7x7 windows over the 112x112 image -> T=256 window tokens, token
  features = (channel, within-window position p) pairs.
  dots[i,j] = <q_i, k_j> * 0.125 ; attn = softmax_j ; out = attn @ v
  y = gelu(out); z = BN(conv1x1(y, wo) + bo); out = gelu(z + x)

Sharding: pure data parallel over batch, 4 images per core on 8 cores.

Implementation notes:
  * The window permute ('b c (h1 ws1) (w1 ws2) -> b (c ws1 ws2) (h1 w1)')
    is done ON THE HOST for the input, and inverted on the host for the
    output: the device sees x and writes out in position-major window
    layout [c, p*T + j], everything contiguous. The HW kernel does zero
    data reshuffling; the only copies are PSUM->SBUF casts.
  * BatchNorms folded into conv weights on the host; SCALE folded into q's
    path; k's bias drops (softmax shift invariance); v's bias passes
    through the attention average into the first gelu's bias; the final
    conv bias + BN fold into the last gelu's bias.
  * q and k never materialize: dots_T[j,i] = sum_p x_pj^T M x_pi with
    M = wk_f^T wq_f precomputed on the host, computed as u_p = M^T x_p
    then dots_T += u_p^T x_p. q's bias contributes a per-row term
    c[j] = sum_p (wk_f^T Bq) . x_p[:,j]; its M=1 matmuls are packed 4-up
    into 32-column strips of the PE array (col tiling), reduced with a
    selector-vector matmul, and added to dots via two rank-1 matmuls.
  * All matmul operands are bf16 (fp32 PSUM accumulation); x_winb is a
    contiguous bf16 copy of the win-layout image made by the otherwise
    idle GPSIMD engine.
  * dots are computed transposed so softmax normalization is a ones-vector
    matmul reduce; no max subtraction needed (|dots| < ~30).
  * The residual add is an identity matmul from x_winb accumulated into
    the out-conv PSUM, so the final gelu reads PSUM directly and writes
    its result IN PLACE into the (dead) x image buffer, contiguous.
"""

import numpy as np

IN_C = 128
HIDE_C = 256
HC2 = 128
OUT_C = 128
WS = 7
SCALE = 0.125
EPS = 1e-5
B, H, W = 32, 112, 112
HW = H * W          # 12544
H1 = H // WS        # 16
W1 = W // WS        # 16
T = H1 * W1         # 256 windows
NP = WS * WS        # 49 positions
NCORES = 8
BPC = B // NCORES   # images per core

F32 = np.float32


def build_bass_kernel(bpc=BPC):
    import concourse.bass as bass
    import concourse.tile as tile
    import concourse.mybir as mybir
    from concourse import bacc

    f32 = mybir.dt.float32
    bf16 = mybir.dt.bfloat16
    fp8 = mybir.dt.float8e4
    DR = mybir.MatmulPerfMode.DoubleRow
    AF = mybir.ActivationFunctionType

    nc = bacc.Bacc("TRN2", target_bir_lowering=False)

    x_d = nc.dram_tensor("x", [bpc, IN_C, HW], f32, kind="ExternalInput")
    m_d = nc.dram_tensor("m", [IN_C, IN_C], bf16, kind="ExternalInput")
    h_d = nc.dram_tensor("hcol", [IN_C, 1], bf16, kind="ExternalInput")
    ident_d = nc.dram_tensor("ident", [128, 128], bf16, kind="ExternalInput")
    wvT_d = nc.dram_tensor("wvT", [IN_C, HIDE_C], bf16, kind="ExternalInput")
    woT_d = nc.dram_tensor("woT", [HIDE_C, OUT_C], bf16, kind="ExternalInput")
    # packed per-partition fp32 bias columns: [Bv_lo, Bv_hi, Bo]
    bias_d = nc.dram_tensor("biases", [128, 3], f32, kind="ExternalInput")
    out_d = nc.dram_tensor("out", [bpc, OUT_C, HW], f32, kind="ExternalOutput")

    # position chunks: (start position, count), raster order
    groups = [(p, 2) for p in range(0, NP - 1, 2)] + [(NP - 1, 1)]

    with tile.TileContext(nc) as tc:
        with (
            tc.tile_pool(name="singles", bufs=1) as singles,
            tc.tile_pool(name="xpool", bufs=2) as xpool,
            tc.tile_pool(name="xwin", bufs=2) as xwin_pool,
            tc.tile_pool(name="u_sb", bufs=4) as u_sb_pool,
            tc.tile_pool(name="v_sb", bufs=12) as v_sb_pool,
            tc.tile_pool(name="g_sb", bufs=3) as g_sb_pool,
            tc.tile_pool(name="attn_sb", bufs=2) as attn_pool,
            tc.tile_pool(name="small_sb", bufs=2) as small_pool,
            tc.tile_pool(name="ps_work", bufs=2, space="PSUM") as ps_work,
            tc.tile_pool(name="ps_dots", bufs=1, space="PSUM") as ps_dots,
            tc.tile_pool(name="ps_av", bufs=3, space="PSUM") as ps_av,
            tc.tile_pool(name="ps_o", bufs=2, space="PSUM") as ps_o_pool,
        ):
            # ---- weights / constants (resident) ----
            m_sb = singles.tile([128, IN_C], bf16)
            nc.sync.dma_start(out=m_sb, in_=m_d.ap())
            h_sb = singles.tile([128, 1], bf16)
            nc.sync.dma_start(out=h_sb, in_=h_d.ap())
            ident = singles.tile([128, 128], bf16)
            nc.sync.dma_start(out=ident, in_=ident_d.ap())
            wvT = singles.tile([128, HIDE_C], bf16)
            nc.sync.dma_start(out=wvT, in_=wvT_d.ap())
            woT = singles.tile([128, 2, OUT_C], bf16)
            nc.sync.dma_start(
                out=woT, in_=woT_d.ap().rearrange("(kc p) m -> p kc m", kc=2)
            )
            biases = singles.tile([128, 3], f32)
            nc.sync.dma_start(out=biases, in_=bias_d.ap())
            bv_ap = [biases[:, 0:1], biases[:, 1:2]]
            bo_ap = biases[:, 2:3]

            ones_mat = singles.tile([128, 128], bf16)
            nc.vector.memset(ones_mat, 1.0)
            ones_row = singles.tile([1, T], bf16)
            nc.vector.memset(ones_row, 1.0)
            sel4 = singles.tile([128, 1], bf16)
            nc.vector.memset(sel4, 0.0)
            for t4 in range(4):
                nc.vector.memset(sel4[32 * t4:32 * t4 + 1, :], 1.0)

            for img in range(bpc):
                # ---- load win-layout x; bf16 copy mostly on GPSIMD ----
                x_img = xpool.tile([128, HW], f32, tag="ximg")
                NLD = 8
                for dc in range(NLD):
                    nc.sync.dma_start(
                        out=x_img[:, dc * (HW // NLD):(dc + 1) * (HW // NLD)],
                        in_=x_d.ap()[img, :, dc * (HW // NLD):(dc + 1) * (HW // NLD)])
                # dummy exp so walrus places the exp ACT-table load here,
                # off the softmax critical chain
                scratch = small_pool.tile([128, 1], f32, tag="scratch")
                nc.scalar.activation(scratch, biases[:, 0:1], AF.Exp)

                x_winb = xwin_pool.tile([128, NP * T], bf16, tag="xwin")
                NCH = 16
                for ch in range(NCH):
                    lo = ch * (HW // NCH)
                    hi = (ch + 1) * (HW // NCH)
                    # GPSIMD casts ~3x slower per element than DVE/ACT, but
                    # is otherwise idle; DVE/ACT take a few chunks to keep
                    # the dots pipeline fed early. For image 0 nothing else
                    # runs yet, so DVE/ACT (fast) do all of it.
                    if img == 0:
                        if ch % 2 == 0:
                            nc.vector.tensor_copy(x_winb[:, lo:hi],
                                                  x_img[:, lo:hi])
                        else:
                            nc.scalar.activation(x_winb[:, lo:hi],
                                                 x_img[:, lo:hi],
                                                 AF.Copy, scale=1.0)
                    elif ch % 8 < 5:
                        nc.gpsimd.tensor_copy(x_winb[:, lo:hi], x_img[:, lo:hi])
                    elif ch % 8 < 7:
                        nc.scalar.activation(x_winb[:, lo:hi], x_img[:, lo:hi],
                                             AF.Copy, scale=1.0)
                    else:
                        nc.vector.tensor_copy(x_winb[:, lo:hi], x_img[:, lo:hi])

                # ---- phase 1: dots_T accumulation over positions ----
                dots_t = ps_dots.tile([128, 512], f32, tag="dots", name="dots")
                dots = [dots_t[:, 0:T], dots_t[:, T:2 * T]]
                chunk_starts = list(range(0, NP, 2))   # 2 positions per chunk
                nchunks = len(chunk_starts)

                def u_conv(ci, p0):
                    npos = min(2, NP - p0)
                    N = npos * T
                    base = p0 * T
                    u_ps = ps_work.tile([128, 512], f32, tag="pwork")
                    nc.tensor.matmul(u_ps[:, :N], lhsT=m_sb,
                                     rhs=x_winb[:, base:base + N],
                                     start=True, stop=True)
                    u_sbt = u_sb_pool.tile([128, 512], bf16, tag="u")
                    nc.vector.tensor_copy(u_sbt[:, :N], u_ps[:, :N])
                    return u_sbt

                def dots_mms(ci, p0, u_sbt):
                    npos = min(2, NP - p0)
                    base = p0 * T
                    first = ci == 0
                    for pi in range(npos):
                        for jh in (0, 1):
                            nc.tensor.matmul(
                                dots[jh],
                                lhsT=u_sbt[:, pi * T + jh * 128:
                                           pi * T + jh * 128 + 128],
                                rhs=x_winb[:, base + pi * T:
                                           base + (pi + 1) * T],
                                start=first and pi == 0 and jh == 0,
                                stop=False,
                                skip_group_check=True)

                pend = []
                for ci, p0 in enumerate(chunk_starts):
                    u_sbt = u_conv(ci, p0)
                    if len(pend) >= 2:
                        dots_mms(*pend.pop(0))
                    pend.append((ci, p0, u_sbt))
                for pe_ in pend:
                    dots_mms(*pe_)
                # c[j] = sum_p h . x_p[:, j]. The M=1 matmuls are packed 4-up
                # into 32-column strips (col tiling), strip t accumulating
                # positions p%4==t concurrently; then a selector-vector
                # matmul reduces the strip rows and two rank-1 matmuls add
                # c into dots. c borrows an out-conv PSUM slot.
                c_row_ps = ps_o_pool.tile([128, 512], f32, tag="ops", name="cps")
                nc.vector.memset(c_row_ps[:, 0:T], 0.0)
                nstrip = [13, 12, 12, 12]
                seen = [0, 0, 0, 0]
                for p in range(NP):
                    t4 = p % 4
                    seen[t4] += 1
                    nc.tensor.matmul(c_row_ps[32 * t4:32 * t4 + 1, 0:T],
                                     lhsT=h_sb,
                                     rhs=x_winb[:, p * T:(p + 1) * T],
                                     start=seen[t4] == 1,
                                     stop=seen[t4] == nstrip[t4],
                                     tile_position=(0, 32 * t4),
                                     skip_group_check=True)
                c_all = small_pool.tile([128, T], bf16, tag="c4sb")
                nc.vector.tensor_copy(c_all, c_row_ps[:, 0:T])
                c_row_ps2 = ps_work.tile([1, T], f32, tag="pwork", name="cps2")
                nc.tensor.matmul(c_row_ps2, lhsT=sel4, rhs=c_all,
                                 start=True, stop=True)
                c_row = small_pool.tile([1, T], bf16, tag="csb")
                nc.vector.tensor_copy(c_row, c_row_ps2)
                for jh in (0, 1):
                    nc.tensor.matmul(
                        dots[jh], lhsT=c_row[:, jh * 128:jh * 128 + 128],
                        rhs=ones_row, start=False, stop=jh == 1,
                        skip_group_check=True)

                # ---- early v-convs (overlap the softmax chain on PE) ----
                def v_conv(g):
                    p0, cnt = g
                    vsb = []
                    for pi in range(cnt):
                        v_ps = ps_work.tile([128, 512], f32, tag="pwork")
                        for jc in (0, 1):
                            nc.tensor.matmul(
                                v_ps[:, jc * HIDE_C:(jc + 1) * HIDE_C],
                                lhsT=x_winb[:, (p0 + pi) * T + jc * 128:
                                            (p0 + pi) * T + jc * 128 + 128],
                                rhs=wvT,
                                start=True, stop=True)
                        v_sbt = v_sb_pool.tile([128, 512], bf16, tag="v")
                        nc.vector.tensor_copy(v_sbt, v_ps)
                        vsb.append(v_sbt)
                    return vsb

                NEARLY = 5
                early_v = [v_conv(g) for g in groups[:NEARLY]]

                # ---- softmax over j (= partitions of dots_T) ----
                attn_e = attn_pool.tile([128, 512], bf16, tag="attne",
                                        name="attne")
                nc.scalar.activation(attn_e, dots_t, AF.Exp)
                s_ps = ps_dots.tile([128, T], f32, tag="dots", name="ssum")
                for jc in (0, 1):
                    nc.tensor.matmul(s_ps, lhsT=ones_mat,
                                     rhs=attn_e[:, jc * T:(jc + 1) * T],
                                     start=jc == 0, stop=jc == 1)
                r_sb = small_pool.tile([128, T], f32, tag="rsb")
                nc.vector.reciprocal_approx_fast(r_sb, s_ps)
                attn2 = attn_pool.tile([128, 512], bf16, tag="attn",
                                       name="attn")
                attn = [attn2[:, 0:T], attn2[:, T:2 * T]]
                for jc in (0, 1):
                    nc.vector.tensor_mul(attn[jc],
                                         attn_e[:, jc * T:(jc + 1) * T], r_sb)

                # ---- phase 2: attention-average, out-conv, residual ----
                next_store = 0
                ST_CH = HW // 8
                vcache = dict(enumerate(early_v))
                for gi, g in enumerate(groups):
                    p0, cnt = g
                    N = cnt * T
                    base = p0 * T
                    if gi + NEARLY < len(groups):
                        vcache[gi + NEARLY] = v_conv(groups[gi + NEARLY])
                    vsb = vcache.pop(gi)

                    g_tiles = []
                    for kc in (0, 1):
                        av = ps_av.tile([128, 512], f32, tag="av", name=f"av{kc}")
                        for pi in range(cnt):
                            for jc in (0, 1):
                                nc.tensor.matmul(
                                    av[:, pi * T:(pi + 1) * T],
                                    lhsT=vsb[pi][:, jc * HIDE_C + kc * 128:
                                                  jc * HIDE_C + kc * 128 + 128],
                                    rhs=attn[jc],
                                    start=jc == 0, stop=jc == 1)
                        g_t = g_sb_pool.tile([128, 512], bf16, tag=f"g{kc}")
                        nc.scalar.activation(g_t[:, :N], av[:, :N], AF.Gelu,
                                             bias=bv_ap[kc], scale=1.0)
                        g_tiles.append(g_t)

                    # out conv + residual identity matmul, accumulated in
                    # PSUM; final gelu reads PSUM, writes the (dead) x
                    # image buffer in place, contiguous win layout
                    o_ps = ps_o_pool.tile([128, 512], f32, tag="ops")
                    for pi in range(cnt):
                        for kc in (0, 1):
                            nc.tensor.matmul(
                                o_ps[:, pi * T:(pi + 1) * T],
                                lhsT=woT[:, kc, :],
                                rhs=g_tiles[kc][:, pi * T:(pi + 1) * T],
                                start=kc == 0, stop=False)
                        nc.tensor.matmul(
                            o_ps[:, pi * T:(pi + 1) * T],
                            lhsT=ident,
                            rhs=x_winb[:, base + pi * T:base + (pi + 1) * T],
                            start=False, stop=True)
                    nc.scalar.activation(x_img[:, base:base + N],
                                         o_ps[:, :N], AF.Gelu,
                                         bias=bo_ap, scale=1.0)

                    # stream the store out as columns finalize: groups write
                    # x_img in raster order, so after group gi the first
                    # base+N columns are final
                    done = base + N
                    while next_store + ST_CH <= done:
                        nc.sync.dma_start(
                            out=out_d.ap()[img, :,
                                           next_store:next_store + ST_CH],
                            in_=x_img[:, next_store:next_store + ST_CH])
                        next_store += ST_CH

    nc.compile()
    return nc


def fold_params(wq, gq, bq, mq, vq, wk, gk, bk, mk, vk,
                wv, gv, bv, mv, vv, wo, bo, go, bbo, mo, vo):
    """Host-side BN/bias folding. Returns (M, h, ident, wvT, woT, biases)."""
    import ml_dtypes
    bf16 = ml_dtypes.bfloat16

    aq = gq / np.sqrt(vq + EPS)
    wq_f = (SCALE * aq)[:, None] * wq
    Bq = SCALE * (bq - aq * mq)

    ak = gk / np.sqrt(vk + EPS)
    wk_f = ak[:, None] * wk          # k bias drops (softmax shift invariance)

    M = wk_f.T @ wq_f                # dots_T = sum_p (M^T x_p)^T x_p
    hv = wk_f.T @ Bq                 # c[j] = sum_p hv . x_p[:, j]

    av = gv / np.sqrt(vv + EPS)
    wv_f = av[:, None] * wv
    Bv = bv - av * mv                # applied inside the first gelu

    ao = go / np.sqrt(vo + EPS)
    wo_f = ao[:, None] * wo
    Bo = ao * (bo - mo) + bbo        # conv bias + BN fold, inside last gelu

    biases = np.stack([Bv[:128], Bv[128:], Bo], axis=1).astype(F32)
    return (np.ascontiguousarray(M).astype(bf16),
            np.ascontiguousarray(hv[:, None]).astype(bf16),
            np.eye(128, dtype=bf16),
            np.ascontiguousarray(wv_f.T).astype(bf16),
            np.ascontiguousarray(wo_f.T).astype(bf16),
            biases)


_CACHED = {}


def _get_nc(bpc=BPC):
    if bpc not in _CACHED:
        _CACHED[bpc] = build_bass_kernel(bpc)
    return _CACHED[bpc]


def _to_win(x):
    """[n, c, H, W] image layout -> [n, c, p*T + j] win layout (host)."""
    n, c = x.shape[:2]
    x = x.reshape(n, c, H1, WS, W1, WS).transpose(0, 1, 3, 5, 2, 4)
    return np.ascontiguousarray(x.reshape(n, c, HW))


def _from_win(y):
    """[n, c, p*T + j] win layout -> [n, c, H, W] image layout (host)."""
    n, c = y.shape[:2]
    y = y.reshape(n, c, WS, WS, H1, W1).transpose(0, 1, 4, 2, 5, 3)
    return y.reshape(n, c, H, W)


def make_in_maps(inputs):
    x = np.asarray(inputs["x"], F32)
    m, hv, ident, wvT, woT, biases = fold_params(
        *[np.asarray(inputs[k], F32) for k in
          ("wq", "gq", "bq", "mq", "vq", "wk", "gk", "bk", "mk", "vk",
           "wv", "gv", "bv", "mv", "vv", "wo", "bo", "go", "bbo", "mo", "vo")]
    )
    in_maps = []
    for c in range(NCORES):
        xs = _to_win(x[c * BPC:(c + 1) * BPC])
        in_maps.append({"x": xs, "m": m, "hcol": hv, "ident": ident,
                        "wvT": wvT, "woT": woT, "biases": biases})
    return in_maps


def kernel(**inputs):
    from concourse.bass_utils import run_bass_kernel_spmd

    in_maps = make_in_maps(inputs)
    nc = _get_nc(BPC)
    res = run_bass_kernel_spmd(nc, in_maps, list(range(NCORES)))
    outs = [_from_win(res.results[c]["out"].reshape(BPC, OUT_C, HW))
            for c in range(NCORES)]
    return np.concatenate(outs, axis=0)
